# revision 47
# baseline (speedup 1.0000x reference)
"""Trainium2 Bass kernel for nn_Attention_14877766713476.

Causal multi-head attention with full-hidden RoPE:
  q,k,v = x@W{q,k,v} + b;  q,k = rope(q,k);  heads=16, hd=128;
  causal softmax attention;  out = attn@Wo + bo.

Sharding: tensor-parallel over heads across 8 cores. RoPE pairs hidden
column c with c +/- 1024, i.e. head h with head h+8 -- so core m owns
heads {m, m+8} and RoPE stays core-local. Each core computes its two
heads end-to-end and a partial output projection (rows of Wo); the host
sums the 8 partials.

All matmuls in bf16 with fp32 PSUM accumulation. Host pre-transposes
x -> xT (contraction dim on partitions) and pre-slices/casts weights,
so the device does zero transposes.

Layouts (per core, host-prepared, all bf16 unless noted):
  xT    [128, 8*16*512] chunk-major: col = (c*16 + a)*512 + t'
        (d = a*128 + p, token t = c*512 + t') -- per-(chunk, g) slices
        are contiguous 4KB lines per partition.
  wq/wk/wv [128, 16*256] col = a*256 + c   (d = a*128 + p, c in 0..255)
  wo    [128, 2*2048]   col = cb*2048 + dcol  (c = cb*128 + p)
  cosT/sinT [128, 4*2*512] chunk-major, batch-deduped:
        col = (cs*2 + cb)*512 + t'  (cs = in-batch chunk 0..3; sinT cb0
        negated so rope_b = q_b*cos_b + q_{1-b}*sinT_b)
  bqk   [128, 4] fp32   cols: bq cb0, bq cb1, bk cb0, bk cb1
  maskb [128, 128]      boundary mask: maskb[kj, q''] = (q'' >= kj)
Output per core: out [4096, 2048] bf16 partial (this core's two heads
through Wo rows); host sums partials in fp32 and adds bv@Wo + bo.

Structure (one core):
  Phase 1 (QKV+RoPE): chunk 0 streams weights (scalar HWDGE ring) and
  x (sync ring) per-g, with q/k/v matmuls chasing the DMAs so PE starts
  ~2us after DMA-go and stays fed through the bandwidth-bound ramp.
  Chunks 1-6 run the plain per-chunk schedule (x prefetched 2 ahead).
  Chunk 7 is emitted as PE-filler units inside phase 2.
  Phase 2 (attention + out-proj): scoresT blocks [kj=128, q=512], exp
  on ScalarE over 2-bank pairs, causal-trimmed on the diagonal (only
  q >= kj-block region computed; dead exp cols zeroed via gpsimd
  memset; single 128-wide boundary mask on DVE). Rowsum via ones-matmul
  over octet pre-sums. PV consumes trimmed expT directly. Out-proj for
  group g interleaves into group g+1's exp-latency bubbles; the final
  group's out-proj runs in a fresh 4-deep PSUM pool with per-512-col
  DMA pieces alternating rings so the drain tail is short.
"""

import math
from contextlib import ExitStack

import numpy as np
import ml_dtypes

N_CORES = 8
B, S, D, H = 2, 2048, 2048, 16
HD = D // H          # 128
T = B * S            # 4096
P = 128
NB = D // P          # 16 contraction blocks
NG = 4               # a-groups (DMA split granularity)
GA = NB // NG        # 4 a-blocks per group
TCH = 512            # token chunk (QKV phase free dim)
NCH = T // TCH       # 8
QBLK = 512           # query block (attention phase free dim)
NQ = S // QBLK       # 4 query blocks per (batch, head)
SCALE = 1.0 / math.sqrt(HD)

BF16 = ml_dtypes.bfloat16

_CACHE = {}
LAST_RESULTS = None


def _build_program():
    import concourse.tile as tile
    from concourse import bacc, mybir

    bf = mybir.dt.bfloat16
    f32 = mybir.dt.float32
    Act = mybir.ActivationFunctionType

    nc = bacc.Bacc("TRN2", target_bir_lowering=False, debug=False,
                   num_devices=N_CORES)

    xT = nc.dram_tensor("xT", [P, NCH * NB * TCH], bf,
                        kind="ExternalInput").ap()
    wq = nc.dram_tensor("wq", [P, NB * 256], bf, kind="ExternalInput").ap()
    wk = nc.dram_tensor("wk", [P, NB * 256], bf, kind="ExternalInput").ap()
    wv = nc.dram_tensor("wv", [P, NB * 256], bf, kind="ExternalInput").ap()
    wo = nc.dram_tensor("wo", [P, 2 * D], bf, kind="ExternalInput").ap()
    cosT = nc.dram_tensor("cosT", [P, 4 * 2 * TCH], bf,
                          kind="ExternalInput").ap()
    sinT = nc.dram_tensor("sinT", [P, 4 * 2 * TCH], bf,
                          kind="ExternalInput").ap()
    bqk = nc.dram_tensor("bqk", [P, 4], f32, kind="ExternalInput").ap()
    maskb = nc.dram_tensor("maskb", [P, P], bf, kind="ExternalInput").ap()
    out = nc.dram_tensor("out", [T, D], bf, kind="ExternalOutput").ap()

    xT5 = xT.rearrange("p (c g a t) -> p c g a t", c=NCH, g=NG, a=GA)
    wq4 = wq.rearrange("p (g a c) -> p g a c", g=NG, a=GA)
    wk4 = wk.rearrange("p (g a c) -> p g a c", g=NG, a=GA)
    wv4 = wv.rearrange("p (g a c) -> p g a c", g=NG, a=GA)
    cosT4 = cosT.rearrange("p (c cb t) -> p c cb t", c=4, cb=2)
    sinT4 = sinT.rearrange("p (c cb t) -> p c cb t", c=4, cb=2)

    with tile.TileContext(nc) as tc, ExitStack() as ctx:
        const = ctx.enter_context(tc.tile_pool(name="const", bufs=1))
        persist = ctx.enter_context(tc.tile_pool(name="persist", bufs=1))
        xt_pool = ctx.enter_context(tc.tile_pool(name="xt", bufs=2))
        cs_pool = ctx.enter_context(tc.tile_pool(name="cs", bufs=2))
        raw_pool = ctx.enter_context(tc.tile_pool(name="raw", bufs=2))
        tmp_pool = ctx.enter_context(tc.tile_pool(name="tmp", bufs=4))
        exp_pool = ctx.enter_context(tc.tile_pool(name="exp", bufs=7))
        rec_pool = ctx.enter_context(tc.tile_pool(name="rec", bufs=2))
        orow_pool = ctx.enter_context(tc.tile_pool(name="orow", bufs=2))

        wq_sb = [const.tile([P, GA, 256], bf, tag=f"wq{g}", name=f"wq_sb{g}")
                 for g in range(NG)]
        wk_sb = [const.tile([P, GA, 256], bf, tag=f"wk{g}", name=f"wk_sb{g}")
                 for g in range(NG)]
        wv_sb = [const.tile([P, GA, 256], bf, tag=f"wv{g}", name=f"wv_sb{g}")
                 for g in range(NG)]
        wo_sb = const.tile([P, 2 * D], bf, tag="wo")
        bqk_sb = const.tile([P, 4], f32, tag="bqk")
        mask_sb = const.tile([P, P], bf, tag="maskb")
        ones_sb = const.tile([P, P], bf, tag="ones")

        # ---- ramp DMA issue order -------------------------------------
        # per-ring throughput caps around ~170 GB/s, so the ramp-critical
        # bytes are balanced across all three rings: sync=x chunk0 (2.1MB),
        # scalar=wq+wv (2MB), gpsimd=wk+consts (1.7MB). First pieces halved
        # so PE starts as early as possible.
        for a_ in range(GA):
            nc.scalar.dma_start(wq_sb[0][:, a_:a_ + 1, :],
                                wq4[:, 0, a_:a_ + 1])
        nc.scalar.dma_start(wv_sb[0][:], wv4[:, 0])
        for g in range(1, NG):
            nc.scalar.dma_start(wq_sb[g][:], wq4[:, g])
            nc.scalar.dma_start(wv_sb[g][:], wv4[:, g])
        # sync ring: the x stream (chunk 0 g0 halved)
        xt0 = [xt_pool.tile([P, GA, TCH], bf, tag=f"xt{g}", name=f"xt{g}_0")
               for g in range(NG)]
        for a_ in range(GA):
            nc.sync.dma_start(xt0[0][:, a_:a_ + 1, :],
                              xT5[:, 0, 0, a_:a_ + 1])
        for g in range(1, NG):
            nc.sync.dma_start(xt0[g][:], xT5[:, 0, g])
        # gpsimd ring (SWDGE): wk stream + small consts; wo deferred
        nc.gpsimd.dma_start(wk_sb[0][:], wk4[:, 0])
        nc.gpsimd.dma_start(wk_sb[1][:], wk4[:, 1])
        nc.gpsimd.dma_start(bqk_sb[:], bqk[:])
        nc.gpsimd.dma_start(wk_sb[2][:], wk4[:, 2])
        nc.gpsimd.dma_start(wk_sb[3][:], wk4[:, 3])
        cos0 = cs_pool.tile([P, 2, TCH], bf, tag="cos", name="cosc_0")
        nc.gpsimd.dma_start(cos0[:], cosT4[:, 0])
        sin0 = cs_pool.tile([P, 2, TCH], bf, tag="sin", name="sinc_0")
        nc.gpsimd.dma_start(sin0[:], sinT4[:, 0])
        nc.gpsimd.dma_start(mask_sb[:], maskb[:])
        # ones on DVE (idle at this point): ready ~6us so the HAM-warmup
        # dummies below can start before any DMA data lands
        nc.vector.memset(ones_sb[:], 1.0)

        # persistent activations
        q_all = persist.tile([P, 2 * T], bf, tag="q_all")      # roped qT
        k_all = persist.tile([P, 2 * T], bf, tag="k_all")      # roped kT
        v_all = persist.tile([P, 32 * 256], bf, tag="v_all")   # v natural
        at_all = persist.tile([P, 2 * T], bf, tag="at_all")    # attnT

        # ---------------- Phase 1: QKV projections + RoPE ----------------
        with tc.tile_pool(name="psum1", bufs=4, space="PSUM") as psum:

            def rope_emit(rawt, dst, cosc, sinc, t0):
                for cb in range(2):
                    tm = tmp_pool.tile([P, TCH], bf, tag="ropetmp")
                    nc.vector.tensor_mul(tm[:], rawt[:, 1 - cb, :],
                                         sinc[:, cb, :])
                    tm2 = tmp_pool.tile([P, TCH], bf, tag="ropetmp2")
                    nc.vector.tensor_mul(tm2[:], rawt[:, cb, :],
                                         cosc[:, cb, :])
                    nc.vector.tensor_add(
                        dst[:, cb * T + t0:cb * T + t0 + TCH], tm[:], tm2[:])

            # -- chunk 0: DMA-chasing schedule. 8 PSUM accumulators open
            # (q/k x cb, v x tt); matmuls for a-group g run as its x and
            # weight tiles land, so PE tracks the bandwidth-bound ramp.
            qk_ps = {}
            for key, wt in (("q", wq_sb), ("k", wk_sb)):
                for cb in range(2):
                    qk_ps[key, cb] = psum.tile([P, TCH], f32, tag="qk",
                                               bufs=4, name=f"c0{key}{cb}")
            v_ps = [psum.tile([P, 256], f32, tag="v", bufs=4,
                              name=f"c0v{tt}") for tt in range(4)]
            # HAM warmup: dummy matmuls on the memset ones tile keep the
            # PE activity monitor busy through the DMA-bound ramp, so the
            # real chunk-0 matmuls run at 2.4 GHz instead of cold 1.2.
            # They borrow a "v" rotation slot; nothing ever reads them.
            warm = psum.tile([P, 256], f32, tag="v", bufs=4, name="warm")
            for _ in range(48):
                nc.tensor.matmul(warm[:, 0:P], ones_sb[:], ones_sb[:],
                                 start=True, stop=True)
            # stage order matches DMA issue order (wq_g, wk_g, wv_g) so the
            # in-order PE queue never head-of-line blocks on a later ring
            # position while earlier-arriving work is ready.
            for g in range(NG):
                for key, wt in (("q", wq_sb), ("k", wk_sb)):
                    for cb in range(2):
                        for a_ in range(GA):
                            a = g * GA + a_
                            nc.tensor.matmul(
                                qk_ps[key, cb][:],
                                wt[g][:, a_, cb * P:cb * P + P],
                                xt0[g][:, a_, :],
                                start=(a == 0), stop=(a == NB - 1))
                for tt in range(4):
                    for a_ in range(GA):
                        a = g * GA + a_
                        nc.tensor.matmul(
                            v_ps[tt][:],
                            xt0[g][:, a_, tt * P:(tt + 1) * P],
                            wv_sb[g][:, a_, :],
                            start=(a == 0), stop=(a == NB - 1))
            qraw0 = raw_pool.tile([P, 2, TCH], bf, tag="qraw", name="qraw_0")
            kraw0 = raw_pool.tile([P, 2, TCH], bf, tag="kraw", name="kraw_0")
            for (key, rawt, bcol) in (("q", qraw0, 0), ("k", kraw0, 2)):
                for cb in range(2):
                    nc.vector.tensor_scalar_add(
                        rawt[:, cb, :], qk_ps[key, cb][:],
                        bqk_sb[:, bcol + cb:bcol + cb + 1])
            for tt in range(4):
                nc.scalar.activation(v_all[:, tt * 256:(tt + 1) * 256],
                                     v_ps[tt][:], Act.Copy)
            rope_emit(qraw0, q_all, cos0, sin0, 0)
            rope_emit(kraw0, k_all, cos0, sin0, 0)

            # -- chunks 1-6: plain per-chunk schedule (x prefetched)
            for tcix in range(1, NCH - 1):
                t0 = tcix * TCH
                xt = [xt_pool.tile([P, GA, TCH], bf, tag=f"xt{g}",
                                   name=f"xt{g}_{tcix}")
                      for g in range(NG)]
                # chunk-1 x rides the gpsimd ring: the sync ring is still
                # draining chunk 0 during the ramp
                xring = nc.gpsimd if tcix == 1 else nc.sync
                for g in range(NG):
                    xring.dma_start(xt[g][:], xT5[:, tcix, g])
                cosc = cs_pool.tile([P, 2, TCH], bf, tag="cos",
                                    name=f"cosc_{tcix}")
                nc.gpsimd.dma_start(cosc[:], cosT4[:, tcix % 4])
                sinc = cs_pool.tile([P, 2, TCH], bf, tag="sin",
                                    name=f"sinc_{tcix}")
                nc.gpsimd.dma_start(sinc[:], sinT4[:, tcix % 4])
                if tcix == 1:
                    nc.gpsimd.dma_start(wo_sb[:], wo[:])

                qraw = raw_pool.tile([P, 2, TCH], bf, tag="qraw",
                                     name=f"qraw_{tcix}")
                kraw = raw_pool.tile([P, 2, TCH], bf, tag="kraw",
                                     name=f"kraw_{tcix}")
                for (wt, rawt, bcol) in ((wq_sb, qraw, 0), (wk_sb, kraw, 2)):
                    for cb in range(2):
                        ps = psum.tile([P, TCH], f32, tag="qk", bufs=4)
                        for a in range(NB):
                            nc.tensor.matmul(
                                ps[:],
                                wt[a // GA][:, a % GA, cb * P:cb * P + P],
                                xt[a // GA][:, a % GA, :],
                                start=(a == 0), stop=(a == NB - 1),
                            )
                        nc.vector.tensor_scalar_add(
                            rawt[:, cb, :], ps[:],
                            bqk_sb[:, bcol + cb:bcol + cb + 1])
                for tt in range(TCH // P):
                    ps = psum.tile([P, 256], f32, tag="v", bufs=4)
                    for a in range(NB):
                        nc.tensor.matmul(
                            ps[:],
                            xt[a // GA][:, a % GA, tt * P:(tt + 1) * P],
                            wv_sb[a // GA][:, a % GA, :],
                            start=(a == 0), stop=(a == NB - 1),
                        )
                    cidx = (tcix * (TCH // P) + tt) * 256
                    nc.scalar.activation(v_all[:, cidx:cidx + 256], ps[:],
                                         Act.Copy)
                rope_emit(qraw, q_all, cosc, sinc, t0)
                rope_emit(kraw, k_all, cosc, sinc, t0)

        # ------- Phase 2+3: causal attention + output projection -------
        # scoresT blocks [kj=128, q=512], diagonal-trimmed; exp on ScalarE
        # over 2-bank pairs; rowsum via ones-matmul over octet pre-sums;
        # PV consumes trimmed expT directly. Software-pipelined (rs/pv
        # trail sc/exp by one pair); out-proj of group g-1 fills PE
        # bubbles inside group g.
        def attn_group(psum, b, cb, qj, late=None):
            qs = cb * T + b * S + qj * QBLK
            nkb = 4 * qj + 4  # key blocks 0..nkb-1
            npairs = nkb // 2
            pv_ps = psum.tile([P, QBLK], f32, tag="pv", bufs=1,
                              name=f"pv_{b}{cb}{qj}")
            rs_ps = psum.tile([P, QBLK], f32, tag="rs", bufs=1,
                              name=f"rs_{b}{cb}{qj}")
            ex_batch = []
            nbatch = (npairs + 3) // 4
            state = {"prev": None, "consumed": 0, "batch": 0}

            def off(i):
                return (i - 4 * qj) * P if i >= 4 * qj else 0

            def flush_rowsum():
                tiles = ex_batch[:]
                ex_batch.clear()
                bi = state["batch"]
                state["batch"] = bi + 1
                parts = []
                for j, e in enumerate(tiles):
                    t = tmp_pool.tile([P, QBLK], bf, tag="esA", bufs=5,
                                      name=f"esA_{b}{cb}{qj}_{bi}_{j}")
                    nc.vector.tensor_add(t[:], e[:, 0, :], e[:, 1, :])
                    parts.append(t)
                li = 0
                while len(parts) > 1:
                    x0 = parts.pop(0)
                    x1 = parts.pop(0)
                    t = tmp_pool.tile([P, QBLK], bf, tag="esB", bufs=3,
                                      name=f"esB_{b}{cb}{qj}_{bi}_{li}")
                    nc.vector.tensor_add(t[:], x0[:], x1[:])
                    parts.append(t)
                    li += 1
                nc.tensor.matmul(rs_ps[:], ones_sb[:], parts[0][:],
                                 start=(bi == 0), stop=(bi == nbatch - 1))

            def consume(ii, ex, defer_flush=False):
                for h in range(2):
                    i = 2 * ii + h
                    o = off(i)
                    if i >= 4 * qj:  # boundary subcol causal mask
                        nc.vector.tensor_mul(
                            ex[:, h, o:o + P], ex[:, h, o:o + P],
                            mask_sb[:, :])
                    vix = (b * 16 + i) * 256 + cb * P
                    nc.tensor.matmul(pv_ps[:, o:QBLK],
                                     v_all[:, vix:vix + P],
                                     ex[:, h, o:QBLK],
                                     start=(i == 0), stop=(i == nkb - 1))
                ex_batch.append(ex)
                state["consumed"] += 1
                if len(ex_batch) == 4 or state["consumed"] == npairs:
                    if defer_flush:
                        state["pending"] = True
                    else:
                        flush_rowsum()

            def pair_step(ii, filler):
                i0, i1 = 2 * ii, 2 * ii + 1
                o0, o1 = off(i0), off(i1)
                diag = i0 >= 4 * qj
                sc_ps = psum.tile([P, 2, QBLK], f32, tag="sc",
                                  name=f"sc_{b}{cb}{qj}_{ii}")
                for h, i, o in ((0, i0, o0), (1, i1, o1)):
                    ks = cb * T + b * S + i * P
                    nc.tensor.matmul(sc_ps[:, h, o:QBLK],
                                     k_all[:, ks:ks + P],
                                     q_all[:, qs + o:qs + QBLK],
                                     start=True, stop=True)
                ex = exp_pool.tile([P, 2, QBLK], bf, tag="exp",
                                   name=f"ex_{b}{cb}{qj}_{ii}")
                if not diag:
                    nc.scalar.activation(ex[:], sc_ps[:], Act.Exp,
                                         scale=SCALE)
                else:
                    # exp only the causal region; zero dead cols (gpsimd,
                    # off the critical engines) so rowsum/PV stay exact
                    nc.scalar.activation(ex[:, :, o0:], sc_ps[:, :, o0:],
                                         Act.Exp, scale=SCALE)
                    if o0 > 0:
                        nc.gpsimd.memset(ex[:, 0, 0:o0], 0)
                    nc.gpsimd.memset(ex[:, 1, 0:o1], 0)
                # independent PE work lands here, between the exp issue and
                # the rs/pv matmuls that wait on it (PE executes in order)
                filler()
                if state["prev"] is not None:
                    consume(*state["prev"])
                state["prev"] = (ii, ex)

            def finish(filler):
                # with `late` units: defer the final rowsum flush so the
                # PE runs the late out-units while DVE sums the exp tree,
                # instead of head-of-line blocking on the ones-matmul
                consume(*state["prev"], defer_flush=late is not None)
                if late is not None:
                    for u in late:
                        u()
                    if state.get("pending"):
                        flush_rowsum()
                rec = rec_pool.tile([P, QBLK], f32, tag="rec",
                                    name=f"rec_{b}{cb}{qj}")
                nc.vector.reciprocal_approx_fast(rec[:], rs_ps[:])
                nc.vector.tensor_mul(at_all[:, qs:qs + QBLK], pv_ps[:],
                                     rec[:])
                filler()

            steps = [(lambda f, ii=ii: pair_step(ii, f))
                     for ii in range(npairs)]
            steps.append(finish)
            return steps

        def out_units(psum, b, qj, fine=False, tail=False):
            # output projection for the 4 token chunks of (b, qj), split
            # into per-(token, dcol) units so they can fill PE bubbles
            # inside the next attention group's exp-chain. fine=True
            # (final group): per-unit DMA pieces, ring-alternated, so the
            # drain after the last matmul is one 128KB transfer.
            units = []

            def unit(tx, dc, orow_box):
                tt = (b * S + qj * QBLK) // P + tx
                if fine:
                    dst_t = orow_pool.tile([P, 512], bf, tag="opiece",
                                           bufs=8, name=f"op_{tt}_{dc}")
                    dst = dst_t[:]
                else:
                    if dc == 0:
                        orow_box.append(orow_pool.tile(
                            [P, D], bf, tag="orow", bufs=4,
                            name=f"orow_{tt}"))
                    orow = orow_box[0]
                    dst = orow[:, dc * 512:(dc + 1) * 512]
                # tail units borrow the sc rotation (dead after the last
                # attention pair) instead of contending with the final
                # interleaved units' out-slot evictions
                if tail:
                    ps = psum.tile([P, 512], f32, tag="sc", bufs=2,
                                   name=f"out_{tt}_{dc}")
                else:
                    ps = psum.tile([P, 512], f32, tag="out",
                                   bufs=4 if fine else 2,
                                   name=f"out_{tt}_{dc}")
                for cb in range(2):
                    nc.tensor.matmul(
                        ps[:],
                        at_all[:, cb * T + tt * P:cb * T + (tt + 1) * P],
                        wo_sb[:, cb * D + dc * 512:cb * D + (dc + 1) * 512],
                        start=(cb == 0), stop=(cb == 1),
                    )
                # alternate eviction engine between ACT and DVE
                if dc % 2 == 0:
                    nc.scalar.activation(dst, ps[:], Act.Copy)
                else:
                    nc.vector.tensor_copy(dst, ps[:])
                if fine:
                    # sync ring only: SWDGE's higher completion latency
                    # would otherwise sit on the end-of-program drain
                    nc.sync.dma_start(
                        out[tt * P:(tt + 1) * P, dc * 512:(dc + 1) * 512],
                        dst_t[:])
                elif dc == D // 512 - 1:
                    ring = nc.sync if tt % 2 == 0 else nc.gpsimd
                    ring.dma_start(out[tt * P:(tt + 1) * P, :], orow[:])

            for tx in range(QBLK // P):
                box = []
                for dc in range(D // 512):
                    units.append(lambda tx=tx, dc=dc, box=box:
                                 unit(tx, dc, box))
            return units

        def chunk7_units(psum):
            # QKV + RoPE for the last token chunk, emitted as PE-filler
            # units inside the b=0 attention section (whose groups don't
            # depend on it). PSUM accumulators borrow the "out" tag slots.
            t0 = (NCH - 1) * TCH
            xt = [xt_pool.tile([P, GA, TCH], bf, tag=f"xt{g}",
                               name=f"xt{g}_7") for g in range(NG)]
            for g in range(NG):
                nc.sync.dma_start(xt[g][:], xT5[:, NCH - 1, g])
            cosc = cs_pool.tile([P, 2, TCH], bf, tag="cos", name="cosc_7")
            nc.gpsimd.dma_start(cosc[:], cosT4[:, 3])
            sinc = cs_pool.tile([P, 2, TCH], bf, tag="sin", name="sinc_7")
            nc.gpsimd.dma_start(sinc[:], sinT4[:, 3])
            qraw = raw_pool.tile([P, 2, TCH], bf, tag="qraw", name="qraw_7")
            kraw = raw_pool.tile([P, 2, TCH], bf, tag="kraw", name="kraw_7")

            def qk_unit(wt, rawt, bcol, cb):
                ps = psum.tile([P, TCH], f32, tag="out",
                               name=f"c7qk_{bcol}{cb}")
                for a in range(NB):
                    nc.tensor.matmul(
                        ps[:], wt[a // GA][:, a % GA, cb * P:cb * P + P],
                        xt[a // GA][:, a % GA, :],
                        start=(a == 0), stop=(a == NB - 1))
                nc.vector.tensor_scalar_add(
                    rawt[:, cb, :], ps[:], bqk_sb[:, bcol + cb:bcol + cb + 1])

            def v_unit(tt):
                ps = psum.tile([P, 256], f32, tag="out", name=f"c7v_{tt}")
                for a in range(NB):
                    nc.tensor.matmul(
                        ps[:], xt[a // GA][:, a % GA, tt * P:(tt + 1) * P],
                        wv_sb[a // GA][:, a % GA, :],
                        start=(a == 0), stop=(a == NB - 1))
                cidx = ((NCH - 1) * (TCH // P) + tt) * 256
                nc.scalar.activation(v_all[:, cidx:cidx + 256], ps[:],
                                     Act.Copy)

            def rope_unit(rawt, dst):
                for cb in range(2):
                    tm = tmp_pool.tile([P, TCH], bf, tag="ropetmp")
                    nc.vector.tensor_mul(tm[:], rawt[:, 1 - cb, :],
                                         sinc[:, cb, :])
                    tm2 = tmp_pool.tile([P, TCH], bf, tag="ropetmp2")
                    nc.vector.tensor_mul(tm2[:], rawt[:, cb, :],
                                         cosc[:, cb, :])
                    nc.vector.tensor_add(
                        dst[:, cb * T + t0:cb * T + t0 + TCH], tm[:], tm2[:])

            return [
                lambda: qk_unit(wq_sb, qraw, 0, 0),
                lambda: qk_unit(wq_sb, qraw, 0, 1),
                lambda: qk_unit(wk_sb, kraw, 2, 0),
                lambda: qk_unit(wk_sb, kraw, 2, 1),
                lambda: v_unit(0), lambda: v_unit(1),
                lambda: v_unit(2), lambda: v_unit(3),
                lambda: rope_unit(qraw, q_all),
                lambda: rope_unit(kraw, k_all),
            ]

        groups = [(b, qj) for b in range(B) for qj in range(NQ)]
        with tc.tile_pool(name="psum2", bufs=2, space="PSUM") as psum:
            c7 = chunk7_units(psum)
            for gi, (b, qj) in enumerate(groups):
                last = gi == len(groups) - 1
                # last group: one token-row of the previous group's out-proj
                # is held back and injected into cb1's finish (between the
                # final PV and the deferred rowsum matmul)
                held = (out_units(psum, *groups[gi - 1], tail=True)[12:]
                        if last else None)
                steps = (attn_group(psum, b, 0, qj)
                         + attn_group(psum, b, 1, qj, late=held))
                outs = out_units(psum, *groups[gi - 1]) if gi >= 1 else []
                if last:
                    outs = outs[:12]
                if gi < 4:  # spread chunk-7 work over the b=0 groups
                    outs = outs + c7[gi * 3:min((gi + 1) * 3, len(c7))]
                k = 0
                for si, st in enumerate(steps):
                    # back-weighted by one step: fillers emitted early delay
                    # the pv/rowsum consumes behind them (measured); late
                    # fillers land in the group-end starve window instead
                    tgt = (len(outs) if si == len(steps) - 1
                           else si * len(outs) // len(steps))

                    def filler(tgt=tgt, outs=outs):
                        nonlocal k
                        while k < tgt:
                            outs[k]()
                            k += 1
                    st(filler)
        # final group's out-proj in a fresh pool: 4-deep PSUM rotation
        # (no eviction-latency stalls) + fine-grained ring-alternated DMA
        with tc.tile_pool(name="psum3", bufs=1, space="PSUM") as psum3:
            for u in out_units(psum3, *groups[-1], fine=True):
                u()

    nc.compile()
    return nc


def _host_prep(x, cos, sin, Wq, bq, Wk, bk, Wv, bv, Wo, bo):
    """Build per-core input maps (numpy, bf16 on-device dtypes)."""
    def pblock(arr, nblk):
        # [nblk*128, F] -> [128, nblk*F] with col = a*F + f
        nb, f = nblk, arr.shape[1]
        return np.ascontiguousarray(
            arr.reshape(nb, P, f).transpose(1, 0, 2).reshape(P, nb * f))

    x2 = np.asarray(x, np.float32).reshape(T, D)
    # chunk-major xT: [p, (c a t')] so per-(chunk, g) slices are contiguous
    xt_ = np.ascontiguousarray(x2.T).reshape(NB, P, NCH, TCH)
    xT_r = np.ascontiguousarray(
        xt_.transpose(1, 2, 0, 3)).reshape(P, NCH * NB * TCH).astype(BF16)

    cosn = np.asarray(cos, np.float32)
    sinn = np.asarray(sin, np.float32)
    Wqn = np.asarray(Wq, np.float32)
    Wkn = np.asarray(Wk, np.float32)
    Wvn = np.asarray(Wv, np.float32)
    Won = np.asarray(Wo, np.float32)
    bqn = np.asarray(bq, np.float32)
    bkn = np.asarray(bk, np.float32)

    # boundary causal mask: maskb[kj, q''] = (q'' >= kj)
    kj = np.arange(P)[:, None]
    qq = np.arange(P)[None, :]
    maskb = (qq - kj >= 0).astype(np.float32)

    common = {
        "xT": xT_r,
        "maskb": maskb.astype(BF16),
    }

    def chunk_major_cs(arr):
        # [256 rows (cb*128+p), S] -> [128, (cs cb t')]
        return np.ascontiguousarray(
            arr.reshape(2, P, 4, TCH).transpose(1, 2, 0, 3)
        ).reshape(P, 4 * 2 * TCH)

    in_maps = []
    for m in range(N_CORES):
        cols = np.r_[128 * m:128 * m + 128, 1024 + 128 * m:1024 + 128 * m + 128]
        wq_s = pblock(Wqn[:, cols], NB).astype(BF16)
        wk_s = pblock(Wkn[:, cols], NB).astype(BF16)
        wv_s = pblock(Wvn[:, cols], NB).astype(BF16)
        wo_s = pblock(Won[cols, :], 2).astype(BF16)

        ct = np.ascontiguousarray(cosn[:, cols].T)     # [256, 2048]
        st = np.ascontiguousarray(sinn[:, cols].T).copy()
        st[:128] *= -1.0                               # sign-fold block0
        cos_s = chunk_major_cs(ct).astype(BF16)
        sin_s = chunk_major_cs(st).astype(BF16)

        bqk_s = np.stack([bqn[cols[:128]], bqn[cols[128:]],
                          bkn[cols[:128]], bkn[cols[128:]]], axis=1)
        bqk_s = np.ascontiguousarray(bqk_s, np.float32)

        in_maps.append(dict(common, wq=wq_s, wk=wk_s, wv=wv_s, wo=wo_s,
                            cosT=cos_s, sinT=sin_s, bqk=bqk_s))
    return in_maps


def kernel(x, cos, sin, Wq, bq, Wk, bk, Wv, bv, Wo, bo):
    global LAST_RESULTS
    from concourse.bass_utils import run_bass_kernel_spmd

    if "nc" not in _CACHE:
        _CACHE["nc"] = _build_program()
    nc = _CACHE["nc"]

    in_maps = _host_prep(x, cos, sin, Wq, bq, Wk, bk, Wv, bv, Wo, bo)
    res = run_bass_kernel_spmd(nc, in_maps, core_ids=list(range(N_CORES)))
    LAST_RESULTS = res

    acc = np.zeros((T, D), np.float32)
    for r in res.results:
        acc += r["out"].astype(np.float32)
    # v-bias and output bias: attn rows sum to 1, so bv contributes bv @ Wo.
    acc += (np.asarray(bv, np.float32) @ np.asarray(Wo, np.float32)
            + np.asarray(bo, np.float32))[None, :]
    return acc.reshape(B, S, D)
